# revision 1
# baseline (speedup 1.0000x reference)
"""DFMConv2d Trainium2 kernel.

Reference computation (per sample b):
  pooled = mean_{h,w} x[b]                          [C=256]
  h      = relu(pooled @ w1.T + b1)                 [128]
  mix    = softmax((h @ w2.T + b2).reshape(256, 8)) [256, 8]
  y      = conv3x3_SAME(x[b], base_filters)         [8, 64, 64]
  out[b] = einsum('on,nhw->ohw', mix, y)            [256, 64, 64]

Strategy (8 NeuronCores, data-parallel over batch, 8 samples/core), all
heavy matmuls in float32r (~2e-4 rel err):

  conv:  y_tap[(t,n), hw] = sum_c filt[t,n,c] * x[c, hw] — all 9 taps in
         the stationary M dim (M=72), so x streams through the PE exactly
         twice; 16 matmuls/sample into a row-padded flat buffer
         ypad[72, 1+66*64+2] (rows -1 and 64 zeroed).
  shift: z[(t,n), hw] = y_tap shifted by (dy-1, dx-1) — one fully
         CONTIGUOUS SBUF->SBUF DMA per tap (offset dy*64+dx into ypad),
         then 6 tiny column-zero fixups for the dx!=1 wraparound cells.
  mix:   out[o, hw] = mixT72.T @ z with K=72; mixT72 = softmax(mix).T
         replicated 9x via 4 doubling partition-shift DMAs.
  DMA issue is split across rings: x loads on GPSIMD/SWDGE, out stores on
  the ACT HWDGE ring, z/mixT/params on the SP ring — avoids FIFO
  head-of-line blocking between pipeline stages.
"""
import sys

sys.path.insert(0, "/opt/trn_rl_repo")

import numpy as np
import ml_dtypes

import concourse.bass as bass
import concourse.bacc as bacc
import concourse.tile as tile
import concourse.mybir as mybir
from concourse.bass_utils import run_bass_kernel_spmd
from contextlib import ExitStack

F32 = mybir.dt.float32
F32R = mybir.dt.float32r
AFT = mybir.ActivationFunctionType
AXX = mybir.AxisListType.X
ALU = mybir.AluOpType

N_CORES = 8
BPC = 8            # samples per core
C = 256
CO = 256
H = W = 64
HW = H * W
NB = 8             # n_base
HID = 128
CCH = 2            # channel chunks of 128
NHC = 8            # h-chunks (8 output rows each)
NT = 9             # taps
M88 = 88           # taps grouped by dx at 32-aligned bases: rows 32*dx+8*dy..+8
YP_LEN = 1 + 66 * 64 + 2   # lead zero + 66 rows + tail slack (reads reach 4225)
TAP_ROW = {(dy, dx): 32 * dx + 8 * dy for dy in range(3) for dx in range(3)}

_BUILT = None


def _build():
    nc = bacc.Bacc("TRN2", target_bir_lowering=False)

    d_x = nc.dram_tensor("x", [BPC, C, HW], F32R, kind="ExternalInput")
    d_w1t = nc.dram_tensor("w1t", [C, HID], F32, kind="ExternalInput")
    d_b1 = nc.dram_tensor("b1", [HID, 1], F32, kind="ExternalInput")
    d_w2p = nc.dram_tensor("w2p", [HID, NB, CO], F32, kind="ExternalInput")
    d_b2t = nc.dram_tensor("b2t", [128, 2, NB], F32, kind="ExternalInput")
    d_ft = nc.dram_tensor("ft", [128, CCH, M88], F32R, kind="ExternalInput")
    d_id = nc.dram_tensor("ident", [128, 128], F32, kind="ExternalInput")
    d_z0 = nc.dram_tensor("zeros", [128, 66], F32R, kind="ExternalInput")
    d_out = nc.dram_tensor("out", [BPC, 2, 128, HW], F32, kind="ExternalOutput")

    with tile.TileContext(nc) as tc, ExitStack() as ctx:
        prm = ctx.enter_context(tc.tile_pool(name="prm", bufs=1))
        xp = ctx.enter_context(tc.tile_pool(name="xp", bufs=2))
        ypp = ctx.enter_context(tc.tile_pool(name="ypp", bufs=2))
        zp = ctx.enter_context(tc.tile_pool(name="zp", bufs=2))
        op = ctx.enter_context(tc.tile_pool(name="op", bufs=3))
        sm = ctx.enter_context(tc.tile_pool(name="sm", bufs=2))
        ps_c = ctx.enter_context(tc.tile_pool(name="ps_c", bufs=2, space="PSUM"))
        ps_m = ctx.enter_context(tc.tile_pool(name="ps_m", bufs=3, space="PSUM"))
        ps_s = ctx.enter_context(tc.tile_pool(name="ps_s", bufs=2, space="PSUM"))

        # ---- params (loaded once) ----
        w1t_sb = prm.tile([128, CCH, HID], F32, tag="w1t")
        nc.sync.dma_start(out=w1t_sb, in_=d_w1t[:, :].rearrange("(cc p) h -> p cc h", p=128))
        b1_sb = prm.tile([128, 1], F32, tag="b1")
        nc.sync.dma_start(out=b1_sb, in_=d_b1[:, :])
        w2p_sb = prm.tile([HID, NB, CO], F32, tag="w2p")
        nc.sync.dma_start(out=w2p_sb, in_=d_w2p[:, :, :])
        b2t_sb = prm.tile([128, 2, NB], F32, tag="b2t")
        nc.sync.dma_start(out=b2t_sb, in_=d_b2t[:, :, :])
        ft_sb = prm.tile([128, CCH, M88], F32R, tag="ft")
        nc.sync.dma_start(out=ft_sb, in_=d_ft[:, :, :])
        id_sb = prm.tile([128, 128], F32, tag="ident")
        nc.sync.dma_start(out=id_sb, in_=d_id[:, :])
        z0_sb = prm.tile([128, 66], F32R, tag="z0")
        nc.sync.dma_start(out=z0_sb, in_=d_z0[:, :])
        pooled_sb = prm.tile([128, CCH, BPC], F32, tag="pooled")
        h_sb = prm.tile([128, BPC], F32, tag="h")
        trash = prm.tile([128, HW], F32, tag="trash")

        for j in range(BPC):
            # ---- load (SWDGE ring) + pooling (split DVE / ACT-accum) ----
            xt = xp.tile([128, CCH, HW], F32R, tag="x")
            nc.gpsimd.dma_start(
                out=xt, in_=d_x[j, :, :].rearrange("(cc p) hw -> p cc hw", p=128))
            nc.vector.reduce_sum(
                pooled_sb[:, 0, j:j + 1], xt[:, 0, :].bitcast(F32), axis=AXX)
            nc.scalar.activation(out=trash, in_=xt[:, 1, :].bitcast(F32),
                                 func=AFT.Copy, accum_out=pooled_sb[:, 1, j:j + 1])

            # ---- attention MLP (fp32) ----
            ph = ps_s.tile([128, 1], F32, tag="sm")
            for cc in range(CCH):
                nc.tensor.matmul(ph, w1t_sb[:, cc, :], pooled_sb[:, cc, j:j + 1],
                                 start=(cc == 0), stop=(cc == 1))
            nc.scalar.activation(out=h_sb[:, j:j + 1], in_=ph, func=AFT.Relu,
                                 bias=b1_sb, scale=1.0)

            mixT_sb = sm.tile([M88, 2, 128], F32R, tag="mixT")
            for oc in range(2):
                pl = ps_s.tile([128, NB], F32, tag="sm")
                for n in range(NB):
                    nc.tensor.matmul(pl[:, n:n + 1],
                                     w2p_sb[:, n, oc * 128:(oc + 1) * 128],
                                     h_sb[:, j:j + 1], start=True, stop=True)
                lg_sb = sm.tile([128, NB], F32, tag="lg_sb")
                nc.vector.tensor_tensor(out=lg_sb, in0=pl, in1=b2t_sb[:, oc, :],
                                        op=ALU.add)
                ex_sb = sm.tile([128, NB], F32, tag="ex_sb")
                nc.scalar.activation(out=ex_sb, in_=lg_sb, func=AFT.Exp)
                sums = sm.tile([128, 1], F32, tag="sums")
                nc.vector.reduce_sum(sums, ex_sb, axis=AXX)
                rec = sm.tile([128, 1], F32, tag="rec")
                nc.vector.reciprocal(rec, sums)
                mix_sb = sm.tile([128, NB], F32, tag="mix_sb")
                nc.vector.tensor_scalar_mul(out=mix_sb, in0=ex_sb, scalar1=rec)
                ptr = ps_s.tile([NB, 128], F32, tag="sm")
                nc.tensor.transpose(ptr, mix_sb, id_sb)
                # DVE cast fp32 -> f32r counts as a rounding producer
                nc.vector.tensor_copy(mixT_sb[0:NB, oc, :], ptr)
            # replicate rows [0:8) nine times via doubling partition-shift DMAs
            nc.sync.dma_start(out=mixT_sb[8:16], in_=mixT_sb[0:8])
            nc.sync.dma_start(out=mixT_sb[16:32], in_=mixT_sb[0:16])
            nc.sync.dma_start(out=mixT_sb[32:64], in_=mixT_sb[0:32])
            nc.sync.dma_start(out=mixT_sb[64:88], in_=mixT_sb[0:24])

            # ---- conv into row-padded flat y_tap ----
            ypad = ypp.tile([M88, YP_LEN], F32R, tag="ypad")
            nc.vector.tensor_copy(ypad[:, 0:65].bitcast(F32),
                                  z0_sb[0:M88, 0:65].bitcast(F32))
            nc.vector.tensor_copy(ypad[:, 4161:4226].bitcast(F32),
                                  z0_sb[0:M88, 0:65].bitcast(F32))
            for hc in range(NHC):
                yps = ps_c.tile([128, 512], F32, tag="yps")
                for cc in range(CCH):
                    nc.tensor.matmul(yps[0:M88, :], ft_sb[:, cc, :],
                                     xt[:, cc, 512 * hc:512 * (hc + 1)],
                                     start=(cc == 0), stop=(cc == 1))
                nc.scalar.copy(
                    out=ypad[:, 65 + 512 * hc:65 + 512 * (hc + 1)].bitcast(F32),
                    in_=yps[0:M88, :])

            # ---- per-tap shifted windows into z (contiguous DMAs) ----
            zt = zp.tile([M88, HW], F32R, tag="z")
            ztv = zt.rearrange("p (h w) -> p h w", w=64)
            for dy in range(3):
                for dx in range(3):
                    r = TAP_ROW[(dy, dx)]
                    off = dy * 64 + dx
                    # dy=2 taps in dx groups 0,1 also copy the zeroed gap rows
                    # (ypad rows r+8..r+16 are zero via the zero filter cols),
                    # so z has no uninitialized rows under the K=88 contraction
                    nr = 16 if (dy == 2 and dx < 2) else NB
                    nc.sync.dma_start(out=zt[r:r + nr, :],
                                      in_=ypad[r:r + nr, off:off + HW])
            # zero the dx wraparound columns: col 0 for dx=0 (rows 0:24),
            # col 63 for dx=2 (rows 64:88)
            nc.vector.tensor_copy(
                ztv[0:24, :, 0:1].rearrange("p h w -> p (h w)"),
                z0_sb[0:24, 0:64].bitcast(F32))
            nc.vector.tensor_copy(
                ztv[64:88, :, 63:64].rearrange("p h w -> p (h w)"),
                z0_sb[64:88, 0:64].bitcast(F32))

            # ---- mix: out[o, hw] = mixT72.T @ z (K=72, f32r) ----
            for oc in range(2):
                ot = op.tile([128, HW], F32, tag="out")
                for hc in range(NHC):
                    om = ps_m.tile([128, 512], F32, tag="ops")
                    nc.tensor.matmul(om, mixT_sb[:, oc, :],
                                     zt[:, 512 * hc:512 * (hc + 1)],
                                     start=True, stop=True)
                    if hc % 2 == 0:
                        nc.vector.tensor_copy(ot[:, 512 * hc:512 * (hc + 1)], om)
                    else:
                        nc.scalar.copy(out=ot[:, 512 * hc:512 * (hc + 1)], in_=om)
                nc.scalar.dma_start(out=d_out[j, oc, :, :], in_=ot)

    nc.compile()
    return nc


def _prep_inputs(x, w1, b1, w2, b2, base_filters):
    """Host-side input layout prep. Returns per-core in_maps."""
    B = x.shape[0]
    xs = np.ascontiguousarray(x.reshape(B, C, HW)).astype(np.float32)
    w1t = np.ascontiguousarray(w1.T).astype(np.float32) / float(HW)
    b1c = np.ascontiguousarray(b1.reshape(HID, 1)).astype(np.float32)
    w2p = np.ascontiguousarray(w2.reshape(CO, NB, HID).transpose(2, 1, 0)).astype(np.float32)
    b2t = np.ascontiguousarray(b2.reshape(2, 128, NB).transpose(1, 0, 2)).astype(np.float32)
    filt = base_filters.reshape(NB, CCH, 128, 3, 3)  # [n, cc, cp, dy, dx]
    # ft[c_part, cc, 32*dx + 8*dy + n] = filt[n, cc, c_part, dy, dx]; gaps zero
    ft = np.zeros((128, CCH, M88), dtype=np.float32)
    for dy in range(3):
        for dx in range(3):
            r = 32 * dx + 8 * dy
            ft[:, :, r:r + NB] = filt[:, :, :, dy, dx].transpose(2, 1, 0)
    ident = np.eye(128, dtype=np.float32)
    zeros = np.zeros((128, 66), dtype=np.float32)

    in_maps = []
    for core in range(N_CORES):
        in_maps.append({
            "x": np.ascontiguousarray(xs[core * BPC:(core + 1) * BPC]),
            "w1t": w1t, "b1": b1c, "w2p": w2p, "b2t": b2t,
            "ft": ft, "ident": ident, "zeros": zeros,
        })
    return in_maps


def kernel(x, w1, b1, w2, b2, base_filters):
    global _BUILT
    if _BUILT is None:
        _BUILT = _build()
    nc = _BUILT
    in_maps = _prep_inputs(np.asarray(x, dtype=np.float32),
                           np.asarray(w1, dtype=np.float32),
                           np.asarray(b1, dtype=np.float32),
                           np.asarray(w2, dtype=np.float32),
                           np.asarray(b2, dtype=np.float32),
                           np.asarray(base_filters, dtype=np.float32))
    res = run_bass_kernel_spmd(nc, in_maps, core_ids=list(range(N_CORES)))
    outs = []
    for core in range(N_CORES):
        o = res.results[core]["out"]            # [BPC, 2, 128, HW]
        outs.append(o.reshape(BPC, CO, H, W))
    return np.concatenate(outs, axis=0).astype(np.float32)



# revision 8
# speedup vs baseline: 1.5628x; 1.5628x over previous
"""DFMConv2d Trainium2 kernel.

Reference computation (per sample b):
  pooled = mean_{h,w} x[b]                          [C=256]
  h      = relu(pooled @ w1.T + b1)                 [128]
  mix    = softmax((h @ w2.T + b2).reshape(256, 8)) [256, 8]
  y      = conv3x3_SAME(x[b], base_filters)         [8, 64, 64]
  out[b] = einsum('on,nhw->ohw', mix, y)            [256, 64, 64]

Strategy (8 NeuronCores, data-parallel over batch, 8 samples/core), heavy
path in bf16 (f32 PSUM accumulation, ~6e-3 rel err):

  conv:  y_tap[(t,n), hw] = sum_c filt[t,n,c] * x[c, hw] — all 9 taps in
         the stationary M dim (M=72 = 9 taps x 8 bases, rows 24*dx+8*dy+n),
         so x streams through the PE exactly twice; 16 matmuls/sample into
         a row-padded flat buffer ypad[72, 1+66*64+2].
  shift: z[(t,n), hw] = y_tap shifted by (dy-1, dx-1) — one contiguous
         SBUF->SBUF DMA per tap (offset dy*64+dx into ypad), plus 2 column
         zero fixups for the dx wraparound cells.
  mix:   out[o, hw] = mixT.T @ z with K=72. The attention MLP + softmax is
         batched over groups of 4 samples; mixT is built without any DMAs:
         softmax output is replicated 9x along the free axis (stride-0
         broadcast read on DVE), then PE-transposed per (sample, oc).
  Engine split: x loads on GPSIMD/SWDGE ring, out stores on the ACT HWDGE
  ring, z shifts on the SP ring. Pooling reduce: cc0 on DVE, cc1 on Pool.
  PSUM drains (ypad + out) alternate ACT(3):DVE(1).
"""
import sys

sys.path.insert(0, "/opt/trn_rl_repo")

import numpy as np
import ml_dtypes

import concourse.bass as bass
import concourse.bacc as bacc
import concourse.tile as tile
import concourse.mybir as mybir
from concourse.bass_utils import run_bass_kernel_spmd
from contextlib import ExitStack

F32 = mybir.dt.float32
BF16 = mybir.dt.bfloat16
AFT = mybir.ActivationFunctionType
AXX = mybir.AxisListType.X
ALU = mybir.AluOpType

N_CORES = 8
BPC = 8            # samples per core
G = 4              # MLP batch group size
C = 256
CO = 256
H = W = 64
HW = H * W
NB = 8             # n_base
HID = 128
CCH = 2            # channel chunks of 128
NHC = 8            # hw-chunks (512 cols each)
NT = 9             # taps
M96 = 96           # taps*bases rows: 32*dx + 8*dy + n (rows 24:32, 56:64, 88:96 zero)
YP_LEN = 1 + 66 * 64 + 2   # lead zero + 66 rows + tail slack
TAP_ROW = {(dy, dx): 32 * dx + 8 * dy for dy in range(3) for dx in range(3)}

_BUILT = None


def _build():
    nc = bacc.Bacc("TRN2", target_bir_lowering=False)

    d_x = nc.dram_tensor("x", [BPC, C, HW], BF16, kind="ExternalInput")
    d_w1t = nc.dram_tensor("w1t", [C, HID], F32, kind="ExternalInput")
    d_b1 = nc.dram_tensor("b1", [HID, 1], F32, kind="ExternalInput")
    d_w2p = nc.dram_tensor("w2p", [HID, NB, CO], BF16, kind="ExternalInput")
    d_b2r = nc.dram_tensor("b2r", [128, 2, G, NB], F32, kind="ExternalInput")
    d_ft = nc.dram_tensor("ft", [128, CCH, M96], BF16, kind="ExternalInput")
    d_id = nc.dram_tensor("ident", [128, 128], BF16, kind="ExternalInput")
    d_z0 = nc.dram_tensor("zeros", [128, 66], BF16, kind="ExternalInput")
    d_out = nc.dram_tensor("out", [BPC, 2, 128, HW], BF16, kind="ExternalOutput")

    with tile.TileContext(nc) as tc, ExitStack() as ctx:
        prm = ctx.enter_context(tc.tile_pool(name="prm", bufs=1))
        xp = ctx.enter_context(tc.tile_pool(name="xp", bufs=3))
        ypp = ctx.enter_context(tc.tile_pool(name="ypp", bufs=2))
        zp = ctx.enter_context(tc.tile_pool(name="zp", bufs=5))
        mt = ctx.enter_context(tc.tile_pool(name="mt", bufs=4))
        op = ctx.enter_context(tc.tile_pool(name="op", bufs=3))
        sm = ctx.enter_context(tc.tile_pool(name="sm", bufs=2))
        ps_c = ctx.enter_context(tc.tile_pool(name="ps_c", bufs=2, space="PSUM"))
        ps_m = ctx.enter_context(tc.tile_pool(name="ps_m", bufs=3, space="PSUM"))
        ps_mlp = ctx.enter_context(tc.tile_pool(name="ps_mlp", bufs=1, space="PSUM"))
        ps_tr = ctx.enter_context(tc.tile_pool(name="ps_tr", bufs=2, space="PSUM"))

        # ---- params (loaded once, SP ring) ----
        w1t_sb = prm.tile([128, CCH, HID], F32, tag="w1t")
        nc.sync.dma_start(out=w1t_sb, in_=d_w1t[:, :].rearrange("(cc p) h -> p cc h", p=128))
        b1_sb = prm.tile([128, 1], F32, tag="b1")
        nc.sync.dma_start(out=b1_sb, in_=d_b1[:, :])
        w2p_sb = prm.tile([HID, NB, CO], BF16, tag="w2p")
        nc.sync.dma_start(out=w2p_sb, in_=d_w2p[:, :, :])
        b2r_sb = prm.tile([128, 2, G, NB], F32, tag="b2r")
        nc.sync.dma_start(out=b2r_sb, in_=d_b2r[:, :, :, :])
        ft_sb = prm.tile([128, CCH, M96], BF16, tag="ft")
        nc.sync.dma_start(out=ft_sb, in_=d_ft[:, :, :])
        id_sb = prm.tile([128, 128], BF16, tag="ident")
        nc.sync.dma_start(out=id_sb, in_=d_id[:, :])
        z0_sb = prm.tile([128, 66], BF16, tag="z0")
        nc.sync.dma_start(out=z0_sb, in_=d_z0[:, :])
        pooled_sb = prm.tile([128, CCH, BPC], F32, tag="pooled")
        h_sb = prm.tile([128, BPC], BF16, tag="h")
        trash = prm.tile([128, HW], BF16, tag="trash")

        xts = {}
        zts = {}
        mixTs = {}
        drain_ctr = [0]

        def drain(out_ap, in_ap):
            # PSUM -> SBUF drains alternate ACT(3) : DVE(1)
            k = drain_ctr[0]
            drain_ctr[0] += 1
            if k % 4 == 3:
                nc.vector.tensor_copy(out_ap, in_ap)
            else:
                nc.scalar.copy(out=out_ap, in_=in_ap)

        def block_load_conv(j):
            xt = xp.tile([128, CCH, HW], BF16, tag="x")
            xts[j] = xt
            nc.gpsimd.dma_start(
                out=xt, in_=d_x[j, :, :].rearrange("(cc p) hw -> p cc hw", p=128))
            # pooling: channel sums (w1t carries the 1/HW scale)
            nc.vector.reduce_sum(pooled_sb[:, 0, j:j + 1], xt[:, 0, :], axis=AXX)
            nc.scalar.activation(out=trash, in_=xt[:, 1, :], func=AFT.Copy,
                                 accum_out=pooled_sb[:, 1, j:j + 1])

            # conv into row-padded flat y_tap
            ypad = ypp.tile([M96, YP_LEN], BF16, tag="ypad")
            nc.gpsimd.tensor_copy(ypad[:, 0:65], z0_sb[0:M96, 0:65])
            nc.gpsimd.tensor_copy(ypad[:, 4161:4226], z0_sb[0:M96, 0:65])
            for hc in range(NHC):
                yps = ps_c.tile([128, 512], F32, tag="yps")
                for cc in range(CCH):
                    nc.tensor.matmul(yps[0:M96, :], ft_sb[:, cc, :],
                                     xt[:, cc, 512 * hc:512 * (hc + 1)],
                                     start=(cc == 0), stop=(cc == 1))
                drain(ypad[:, 65 + 512 * hc:65 + 512 * (hc + 1)], yps[0:M96, :])

            # per-tap shifted windows into z (contiguous SBUF->SBUF DMAs)
            zt = zp.tile([M96, HW], BF16, tag="z")
            zts[j] = zt
            for dy in range(3):
                for dx in range(3):
                    r = TAP_ROW[(dy, dx)]
                    off = dy * 64 + dx
                    nr = 16 if dy == 2 else NB
                    nc.sync.dma_start(out=zt[r:r + nr, :],
                                      in_=ypad[r:r + nr, off:off + HW])
            # zero the dx wraparound columns: col 0 for dx=0, col 63 for dx=2
            ztv = zt.rearrange("p (h w) -> p h w", w=64)
            nc.vector.tensor_copy(
                ztv[0:24, :, 0:1].rearrange("p h w -> p (h w)"),
                z0_sb[0:24, 0:64])
            nc.vector.tensor_copy(
                ztv[64:88, :, 63:64].rearrange("p h w -> p (h w)"),
                z0_sb[64:88, 0:64])

        def block_mlp(g):
            j0 = G * g
            # MLP psum: one bank shared by layer-1 (cols 64:68) and layer-2 (0:64)
            pmlp = ps_mlp.tile([128, 128], F32, tag="pmlp")
            ph = pmlp[:, 64:64 + G]
            # MLP layer 1 (f32): h = relu(W1 @ pooled + b1) for 4 samples
            for cc in range(CCH):
                nc.tensor.matmul(ph, w1t_sb[:, cc, :], pooled_sb[:, cc, j0:j0 + G],
                                 start=(cc == 0), stop=(cc == 1))
            nc.scalar.activation(out=h_sb[:, j0:j0 + G], in_=ph, func=AFT.Relu,
                                 bias=b1_sb, scale=1.0)
            # MLP layer 2 (bf16): logits[o, oc, smp, n]
            pl = pmlp[:, 0:64].rearrange("p (oc g n) -> p oc g n", oc=2, g=G)
            for oc in range(2):
                for n in range(NB):
                    nc.tensor.matmul(pl[:, oc, :, n],
                                     w2p_sb[:, n, oc * 128:(oc + 1) * 128],
                                     h_sb[:, j0:j0 + G], start=True, stop=True)
            lg = sm.tile([128, 2, G, NB], F32, tag="lg")
            nc.vector.tensor_tensor(out=lg, in0=pl, in1=b2r_sb, op=ALU.add)
            ex = sm.tile([128, 2, G, NB], F32, tag="ex")
            nc.scalar.activation(out=ex, in_=lg, func=AFT.Exp)
            sums = sm.tile([128, 2, G], F32, tag="sums")
            nc.vector.reduce_sum(sums, ex, axis=AXX)
            rec = sm.tile([128, 2, G], F32, tag="rec")
            nc.vector.reciprocal(rec, sums)
            # normalized softmax replicated 9x along free axis (one DVE op)
            mixrep = sm.tile([128, 2, G, 12, NB], BF16, tag="mixrep")
            for oc in range(2):
                nc.vector.tensor_tensor(
                    out=mixrep[:, oc],
                    in0=ex[:, oc].unsqueeze(2).to_broadcast([128, G, 12, NB]),
                    in1=rec[:, oc].unsqueeze(2).unsqueeze(3).to_broadcast(
                        [128, G, 12, NB]),
                    op=ALU.mult)
            # mixT[(t,n), oc, o] via PE transpose per (sample, oc)
            for jj in range(G):
                mixT = mt.tile([M96, 2, 128], BF16, tag="mixT")
                mixTs[j0 + jj] = mixT
                for oc in range(2):
                    ptr = ps_tr.tile([M96, 128], BF16, tag="ptr")
                    nc.tensor.transpose(
                        ptr, mixrep[:, oc, jj, :, :].rearrange("p a b -> p (a b)"),
                        id_sb)
                    nc.vector.tensor_copy(mixT[:, oc, :], ptr)

        def block_mix(j):
            zt = zts.pop(j)
            mixT = mixTs.pop(j)
            for oc in range(2):
                ot = op.tile([128, HW], BF16, tag="out")
                for hc in range(NHC):
                    om = ps_m.tile([128, 512], F32, tag="om")
                    nc.tensor.matmul(om, mixT[:, oc, :],
                                     zt[:, 512 * hc:512 * (hc + 1)],
                                     start=True, stop=True)
                    drain(ot[:, 512 * hc:512 * (hc + 1)], om)
                nc.scalar.dma_start(out=d_out[j, oc, :, :], in_=ot)

        for j in range(BPC):
            block_load_conv(j)
            if j >= G:
                block_mix(j - G)
            if j == G - 1:
                block_mlp(0)
        block_mlp(1)
        for j in range(G, BPC):
            block_mix(j)

    nc.compile()
    return nc


def _prep_inputs(x, w1, b1, w2, b2, base_filters):
    """Host-side input layout prep. Returns per-core in_maps."""
    B = x.shape[0]
    xs = np.ascontiguousarray(x.reshape(B, C, HW)).astype(ml_dtypes.bfloat16)
    w1t = np.ascontiguousarray(w1.T).astype(np.float32) / float(HW)
    b1c = np.ascontiguousarray(b1.reshape(HID, 1)).astype(np.float32)
    w2p = np.ascontiguousarray(
        w2.reshape(CO, NB, HID).transpose(2, 1, 0)).astype(ml_dtypes.bfloat16)
    # b2r[o_part, oc, smp, n] = b2[(oc*128 + o_part)*8 + n]
    b2r = np.broadcast_to(
        b2.reshape(2, 128, NB).transpose(1, 0, 2)[:, :, None, :],
        (128, 2, G, NB))
    b2r = np.ascontiguousarray(b2r).astype(np.float32)
    filt = base_filters.reshape(NB, CCH, 128, 3, 3)  # [n, cc, cp, dy, dx]
    # ft[c_part, cc, 24*dx + 8*dy + n] = filt[n, cc, c_part, dy, dx]
    ft = np.zeros((128, CCH, M96), dtype=np.float32)
    for dy in range(3):
        for dx in range(3):
            r = TAP_ROW[(dy, dx)]
            ft[:, :, r:r + NB] = filt[:, :, :, dy, dx].transpose(2, 1, 0)
    ft = ft.astype(ml_dtypes.bfloat16)
    ident = np.eye(128, dtype=np.float32).astype(ml_dtypes.bfloat16)
    zeros = np.zeros((128, 66), dtype=ml_dtypes.bfloat16)

    in_maps = []
    for core in range(N_CORES):
        in_maps.append({
            "x": np.ascontiguousarray(xs[core * BPC:(core + 1) * BPC]),
            "w1t": w1t, "b1": b1c, "w2p": w2p, "b2r": b2r,
            "ft": ft, "ident": ident, "zeros": zeros,
        })
    return in_maps


def kernel(x, w1, b1, w2, b2, base_filters):
    global _BUILT
    if _BUILT is None:
        _BUILT = _build()
    nc = _BUILT
    in_maps = _prep_inputs(np.asarray(x, dtype=np.float32),
                           np.asarray(w1, dtype=np.float32),
                           np.asarray(b1, dtype=np.float32),
                           np.asarray(w2, dtype=np.float32),
                           np.asarray(b2, dtype=np.float32),
                           np.asarray(base_filters, dtype=np.float32))
    res = run_bass_kernel_spmd(nc, in_maps, core_ids=list(range(N_CORES)))
    outs = []
    for core in range(N_CORES):
        o = np.asarray(res.results[core]["out"])    # [BPC, 2, 128, HW] bf16
        outs.append(o.reshape(BPC, CO, H, W).astype(np.float32))
    return np.concatenate(outs, axis=0)


# revision 9
# speedup vs baseline: 1.7557x; 1.1234x over previous
"""DFMConv2d Trainium2 kernel.

Reference computation (per sample b):
  pooled = mean_{h,w} x[b]                          [C=256]
  h      = relu(pooled @ w1.T + b1)                 [128]
  mix    = softmax((h @ w2.T + b2).reshape(256, 8)) [256, 8]
  y      = conv3x3_SAME(x[b], base_filters)         [8, 64, 64]
  out[b] = einsum('on,nhw->ohw', mix, y)            [256, 64, 64]

Strategy (8 NeuronCores, data-parallel over batch, 8 samples/core), heavy
path in bf16 (f32 PSUM accumulation, ~6e-3 rel err):

  conv:  y_tap[(t,n), hw] = sum_c filt[t,n,c] * x[c, hw] — all 9 taps in
         the stationary M dim (M=72 = 9 taps x 8 bases, rows 24*dx+8*dy+n),
         so x streams through the PE exactly twice; 16 matmuls/sample into
         a row-padded flat buffer ypad[72, 1+66*64+2].
  shift: z[(t,n), hw] = y_tap shifted by (dy-1, dx-1) — one contiguous
         SBUF->SBUF DMA per tap (offset dy*64+dx into ypad), plus 2 column
         zero fixups for the dx wraparound cells.
  mix:   out[o, hw] = mixT.T @ z with K=72. The attention MLP + softmax is
         batched over groups of 4 samples; mixT is built without any DMAs:
         softmax output is replicated 9x along the free axis (stride-0
         broadcast read on DVE), then PE-transposed per (sample, oc).
  Engine split: x loads on GPSIMD/SWDGE ring, out stores on the ACT HWDGE
  ring, z shifts on the SP ring. Pooling reduce: cc0 on DVE, cc1 on Pool.
  PSUM drains (ypad + out) alternate ACT(3):DVE(1).
"""
import sys

sys.path.insert(0, "/opt/trn_rl_repo")

import numpy as np
import ml_dtypes

import concourse.bass as bass
import concourse.bacc as bacc
import concourse.tile as tile
import concourse.mybir as mybir
from concourse.bass_utils import run_bass_kernel_spmd
from contextlib import ExitStack

F32 = mybir.dt.float32
BF16 = mybir.dt.bfloat16
AFT = mybir.ActivationFunctionType
AXX = mybir.AxisListType.X
ALU = mybir.AluOpType

N_CORES = 8
BPC = 8            # samples per core
G = 4              # MLP batch group size
C = 256
CO = 256
H = W = 64
HW = H * W
NB = 8             # n_base
HID = 128
CCH = 2            # channel chunks of 128
NHC = 8            # hw-chunks (512 cols each)
NT = 9             # taps
M96 = 96           # taps*bases rows: 32*dx + 8*dy + n (rows 24:32, 56:64, 88:96 zero)
YP_LEN = 1 + 66 * 64 + 2   # lead zero + 66 rows + tail slack
TAP_ROW = {(dy, dx): 32 * dx + 8 * dy for dy in range(3) for dx in range(3)}

_BUILT = None


def _build():
    nc = bacc.Bacc("TRN2", target_bir_lowering=False)

    d_x = nc.dram_tensor("x", [BPC, C, HW], BF16, kind="ExternalInput")
    d_w1t = nc.dram_tensor("w1t", [C, HID], F32, kind="ExternalInput")
    d_b1 = nc.dram_tensor("b1", [HID, 1], F32, kind="ExternalInput")
    d_w2p = nc.dram_tensor("w2p", [HID, NB, CO], BF16, kind="ExternalInput")
    d_b2r = nc.dram_tensor("b2r", [128, 2, G, NB], F32, kind="ExternalInput")
    d_ft = nc.dram_tensor("ft", [128, CCH, M96], BF16, kind="ExternalInput")
    d_id = nc.dram_tensor("ident", [128, 128], BF16, kind="ExternalInput")
    d_z0 = nc.dram_tensor("zeros", [128, 66], BF16, kind="ExternalInput")
    d_out = nc.dram_tensor("out", [BPC, 2, 128, HW], BF16, kind="ExternalOutput")

    with tile.TileContext(nc) as tc, ExitStack() as ctx:
        prm = ctx.enter_context(tc.tile_pool(name="prm", bufs=1))
        xp = ctx.enter_context(tc.tile_pool(name="xp", bufs=3))
        ypp = ctx.enter_context(tc.tile_pool(name="ypp", bufs=2))
        zp = ctx.enter_context(tc.tile_pool(name="zp", bufs=5))
        mt = ctx.enter_context(tc.tile_pool(name="mt", bufs=4))
        op = ctx.enter_context(tc.tile_pool(name="op", bufs=3))
        sm = ctx.enter_context(tc.tile_pool(name="sm", bufs=2))
        ps_c = ctx.enter_context(tc.tile_pool(name="ps_c", bufs=2, space="PSUM"))
        ps_m = ctx.enter_context(tc.tile_pool(name="ps_m", bufs=2, space="PSUM"))

        # ---- params (loaded once, SP ring) ----
        w1t_sb = prm.tile([128, CCH, HID], F32, tag="w1t")
        nc.sync.dma_start(out=w1t_sb, in_=d_w1t[:, :].rearrange("(cc p) h -> p cc h", p=128))
        b1_sb = prm.tile([128, 1], F32, tag="b1")
        nc.sync.dma_start(out=b1_sb, in_=d_b1[:, :])
        w2p_sb = prm.tile([HID, NB, CO], BF16, tag="w2p")
        nc.sync.dma_start(out=w2p_sb, in_=d_w2p[:, :, :])
        b2r_sb = prm.tile([128, 2, G, NB], F32, tag="b2r")
        nc.sync.dma_start(out=b2r_sb, in_=d_b2r[:, :, :, :])
        ft_sb = prm.tile([128, CCH, M96], BF16, tag="ft")
        nc.sync.dma_start(out=ft_sb, in_=d_ft[:, :, :])
        id_sb = prm.tile([128, 128], BF16, tag="ident")
        nc.sync.dma_start(out=id_sb, in_=d_id[:, :])
        z0_sb = prm.tile([128, 66], BF16, tag="z0")
        nc.sync.dma_start(out=z0_sb, in_=d_z0[:, :])
        pooled_sb = prm.tile([128, CCH, BPC], F32, tag="pooled")
        h_sb = prm.tile([128, BPC], BF16, tag="h")
        pls = ctx.enter_context(tc.tile_pool(name="pls", bufs=2))

        xts = {}
        zts = {}
        mixTs = {}
        drain_ctr = [0]

        def drain(out_ap, in_ap):
            # PSUM -> SBUF drains alternate ACT(7) : DVE(5)
            k = drain_ctr[0] % 12
            drain_ctr[0] += 1
            if k in (0, 2, 4, 6, 8):
                nc.vector.tensor_copy(out_ap, in_ap)
            else:
                nc.scalar.copy(out=out_ap, in_=in_ap)

        def block_load(j):
            xt = xp.tile([128, CCH, HW], BF16, tag="x")
            xts[j] = xt
            nc.gpsimd.dma_start(
                out=xt, in_=d_x[j, :, :].rearrange("(cc p) hw -> p cc hw", p=128))

        def block_conv(j):
            xt = xts[j]
            # pooling: halving add on Pool engine, then DVE reduce per chunk
            # (w1t carries the 1/HW scale)
            tmp = pls.tile([128, CCH, HW // 2], BF16, tag="ptree")
            nc.gpsimd.tensor_tensor(out=tmp[:, 0], in0=xt[:, 0, 0:HW // 2],
                                    in1=xt[:, 0, HW // 2:HW], op=ALU.add)
            nc.gpsimd.tensor_tensor(out=tmp[:, 1], in0=xt[:, 1, 0:HW // 2],
                                    in1=xt[:, 1, HW // 2:HW], op=ALU.add)
            nc.vector.reduce_sum(pooled_sb[:, 0, j:j + 1], tmp[:, 0], axis=AXX)
            nc.vector.reduce_sum(pooled_sb[:, 1, j:j + 1], tmp[:, 1], axis=AXX)

            # conv into row-padded flat y_tap
            ypad = ypp.tile([M96, YP_LEN], BF16, tag="ypad")
            nc.gpsimd.tensor_copy(ypad[:, 0:65], z0_sb[0:M96, 0:65])
            nc.gpsimd.tensor_copy(ypad[:, 4161:4226], z0_sb[0:M96, 0:65])
            for hc in range(NHC // 2):
                yps = ps_c.tile([128, 1024], F32, tag="yps")
                for half in range(2):
                    for cc in range(CCH):
                        c0 = 1024 * hc + 512 * half
                        nc.tensor.matmul(yps[0:M96, 512 * half:512 * (half + 1)],
                                         ft_sb[:, cc, :], xt[:, cc, c0:c0 + 512],
                                         start=(cc == 0), stop=(cc == 1))
                drain(ypad[:, 65 + 1024 * hc:65 + 1024 * (hc + 1)], yps[0:M96, :])

            # per-tap shifted windows into z (contiguous SBUF->SBUF DMAs)
            zt = zp.tile([M96, HW], BF16, tag="z")  # noqa: continues block_conv
            zts[j] = zt
            for dy in range(3):
                for dx in range(3):
                    r = TAP_ROW[(dy, dx)]
                    off = dy * 64 + dx
                    nr = 16 if dy == 2 else NB
                    nc.sync.dma_start(out=zt[r:r + nr, :],
                                      in_=ypad[r:r + nr, off:off + HW])
            # zero the dx wraparound columns: col 0 for dx=0, col 63 for dx=2
            ztv = zt.rearrange("p (h w) -> p h w", w=64)
            nc.vector.tensor_copy(
                ztv[0:24, :, 0:1].rearrange("p h w -> p (h w)"),
                z0_sb[0:24, 0:64])
            nc.vector.tensor_copy(
                ztv[64:88, :, 63:64].rearrange("p h w -> p (h w)"),
                z0_sb[64:88, 0:64])

        def block_mlp(g):
            j0 = G * g
            # MLP psum: borrow one mix-pool slot; layer-1 at cols 64:68,
            # layer-2 at 0:64, transposes at 128:640 (8 bf16 regions of 64)
            pmlp = ps_m.tile([128, 1024], F32, tag="om")
            ph = pmlp[:, 64:64 + G]
            # MLP layer 1 (f32): h = relu(W1 @ pooled + b1) for 4 samples
            for cc in range(CCH):
                nc.tensor.matmul(ph, w1t_sb[:, cc, :], pooled_sb[:, cc, j0:j0 + G],
                                 start=(cc == 0), stop=(cc == 1))
            nc.scalar.activation(out=h_sb[:, j0:j0 + G], in_=ph, func=AFT.Relu,
                                 bias=b1_sb, scale=1.0)
            # MLP layer 2 (bf16): logits[o, oc, smp, n]
            pl = pmlp[:, 0:64].rearrange("p (oc g n) -> p oc g n", oc=2, g=G)
            ptr_base = 128
            for oc in range(2):
                for n in range(NB):
                    nc.tensor.matmul(pl[:, oc, :, n],
                                     w2p_sb[:, n, oc * 128:(oc + 1) * 128],
                                     h_sb[:, j0:j0 + G], start=True, stop=True)
            lg = sm.tile([128, 2, G, NB], F32, tag="lg")
            nc.vector.tensor_tensor(out=lg, in0=pl, in1=b2r_sb, op=ALU.add)
            ex = sm.tile([128, 2, G, NB], F32, tag="ex")
            nc.scalar.activation(out=ex, in_=lg, func=AFT.Exp)
            sums = sm.tile([128, 2, G], F32, tag="sums")
            nc.vector.reduce_sum(sums, ex, axis=AXX)
            rec = sm.tile([128, 2, G], F32, tag="rec")
            nc.vector.reciprocal(rec, sums)
            # normalized softmax replicated 9x along free axis (one DVE op)
            mixrep = sm.tile([128, 2, G, 12, NB], BF16, tag="mixrep")
            for oc in range(2):
                nc.vector.tensor_tensor(
                    out=mixrep[:, oc],
                    in0=ex[:, oc].unsqueeze(2).to_broadcast([128, G, 12, NB]),
                    in1=rec[:, oc].unsqueeze(2).unsqueeze(3).to_broadcast(
                        [128, G, 12, NB]),
                    op=ALU.mult)
            # mixT[(t,n), oc, o] via PE transpose per (sample, oc)
            for jj in range(G):
                mixT = mt.tile([M96, 2, 128], BF16, tag="mixT")
                mixTs[j0 + jj] = mixT
                for oc in range(2):
                    k = 2 * jj + oc
                    ptr = pmlp[0:M96, ptr_base + 64 * k:ptr_base + 64 * (k + 1)]
                    ptr = ptr.bitcast(BF16)
                    nc.tensor.transpose(
                        ptr, mixrep[:, oc, jj, :, :].rearrange("p a b -> p (a b)"),
                        id_sb)
                    nc.vector.tensor_copy(mixT[:, oc, :], ptr)

        def block_mix(j):
            zt = zts.pop(j)
            mixT = mixTs.pop(j)
            for oc in range(2):
                ot = op.tile([128, HW], BF16, tag="out")
                for hc in range(NHC // 2):
                    om = ps_m.tile([128, 1024], F32, tag="om")
                    for half in range(2):
                        c0 = 1024 * hc + 512 * half
                        nc.tensor.matmul(om[:, 512 * half:512 * (half + 1)],
                                         mixT[:, oc, :], zt[:, c0:c0 + 512],
                                         start=True, stop=True)
                    drain(ot[:, 1024 * hc:1024 * (hc + 1)], om)
                nc.scalar.dma_start(out=d_out[j, oc, :, :], in_=ot)

        for j in range(BPC):
            block_load(j)
            if j >= G:
                block_mix(j - G)
            block_conv(j)
            if j == G - 1:
                block_mlp(0)
        block_mlp(1)
        for j in range(G, BPC):
            block_mix(j)

    nc.compile()
    return nc


def _prep_inputs(x, w1, b1, w2, b2, base_filters):
    """Host-side input layout prep. Returns per-core in_maps."""
    B = x.shape[0]
    xs = np.ascontiguousarray(x.reshape(B, C, HW)).astype(ml_dtypes.bfloat16)
    w1t = np.ascontiguousarray(w1.T).astype(np.float32) / float(HW)
    b1c = np.ascontiguousarray(b1.reshape(HID, 1)).astype(np.float32)
    w2p = np.ascontiguousarray(
        w2.reshape(CO, NB, HID).transpose(2, 1, 0)).astype(ml_dtypes.bfloat16)
    # b2r[o_part, oc, smp, n] = b2[(oc*128 + o_part)*8 + n]
    b2r = np.broadcast_to(
        b2.reshape(2, 128, NB).transpose(1, 0, 2)[:, :, None, :],
        (128, 2, G, NB))
    b2r = np.ascontiguousarray(b2r).astype(np.float32)
    filt = base_filters.reshape(NB, CCH, 128, 3, 3)  # [n, cc, cp, dy, dx]
    # ft[c_part, cc, 24*dx + 8*dy + n] = filt[n, cc, c_part, dy, dx]
    ft = np.zeros((128, CCH, M96), dtype=np.float32)
    for dy in range(3):
        for dx in range(3):
            r = TAP_ROW[(dy, dx)]
            ft[:, :, r:r + NB] = filt[:, :, :, dy, dx].transpose(2, 1, 0)
    ft = ft.astype(ml_dtypes.bfloat16)
    ident = np.eye(128, dtype=np.float32).astype(ml_dtypes.bfloat16)
    zeros = np.zeros((128, 66), dtype=ml_dtypes.bfloat16)

    in_maps = []
    for core in range(N_CORES):
        in_maps.append({
            "x": np.ascontiguousarray(xs[core * BPC:(core + 1) * BPC]),
            "w1t": w1t, "b1": b1c, "w2p": w2p, "b2r": b2r,
            "ft": ft, "ident": ident, "zeros": zeros,
        })
    return in_maps


def kernel(x, w1, b1, w2, b2, base_filters):
    global _BUILT
    if _BUILT is None:
        _BUILT = _build()
    nc = _BUILT
    in_maps = _prep_inputs(np.asarray(x, dtype=np.float32),
                           np.asarray(w1, dtype=np.float32),
                           np.asarray(b1, dtype=np.float32),
                           np.asarray(w2, dtype=np.float32),
                           np.asarray(b2, dtype=np.float32),
                           np.asarray(base_filters, dtype=np.float32))
    res = run_bass_kernel_spmd(nc, in_maps, core_ids=list(range(N_CORES)))
    outs = []
    for core in range(N_CORES):
        o = np.asarray(res.results[core]["out"])    # [BPC, 2, 128, HW] bf16
        outs.append(o.reshape(BPC, CO, H, W).astype(np.float32))
    return np.concatenate(outs, axis=0)
